# revision 7
# baseline (speedup 1.0000x reference)
"""Trainium2 Bass kernel for nn_Attention1 (dense transformer attention with
query-summed output).

Reference computation (per batch b):
    query  = x * drop_mask                       [S, D]
    scores = query @ x.T / sqrt(D)               [S, S]
    att    = softmax(scores, axis=-1)
    out[b] = (att @ x).sum(axis=queries)         [D]

Key identity: out[b] = w @ x where w[k] = sum_q att[q, k] (attention column
sums), so the full PV matmul is never needed — only the column sums of the
softmax matrix.

Sharding: pure data parallel, batch B=8 across the 8 NeuronCores.

v2 design (per core, S=4096, D=256):
  Phase A (all-DMA front end, no engine deps on the critical path):
    - SWDGE cast-DMAs f32->f16 to DRAM bounce (x16d, m16d), chunked.
    - XBAR transpose loads on both HWDGE rings: xT16 on sync, mT16 on scalar.
    - Plain load x16 (row layout, for the final matvec).
    - DVE: qT8 = (xT16/4) * mT16, xT8 = xT16/4  (fp8e4, so s = q.x/16).
    - DVE: dprod = qT8 * xT8 (fp16); PE ones-matvecs + K=1 transpose matmuls
      give the per-query diagonal scores -> bias_all = 8*ln2 - diag.
  Phase B (32 stripes of 128 queries):
    - fp8 DoubleRow matmuls (K=256 folded into one MM): 8 MMs of N=512/stripe.
    - ScalarE stripes: EXP activation with per-row bias + fused row-sum accum.
    - DVE stripes: bit-trick exp: uint16 <- round(sat(s*C1 + bias2)), the
      uint16 pattern IS the fp16 e value (negatives saturate to 0 = +0.0).
      Row sums via a 4x-mode dummy tensor_scalar with accum_out.
    - Column sums w += r_q e[q,:] as M=1 matvecs (4 col-strips packed per PSUM
      bank via tile_position), accumulated in two persistent PSUM banks across
      all 32 stripes.
  Tail: W banks -> SBUF, 32 K=1 transpose MMs -> w in partition layout,
    out = w16 @ x16 (32 accumulating fp16 MMs), copy, DMA out.
"""

import os
import sys

import numpy as np

_TRN_REPO = "/opt/trn_rl_repo"
if os.path.isdir(_TRN_REPO) and _TRN_REPO not in sys.path:
    sys.path.insert(0, _TRN_REPO)

import concourse.bass as bass
import concourse.mybir as mybir
import concourse.tile as tile
from concourse import bacc
from concourse.bass_utils import run_bass_kernel_spmd

F32 = mybir.dt.float32
F16 = mybir.dt.float16
U16 = mybir.dt.uint16
F8 = mybir.dt.float8e4
DR = mybir.MatmulPerfMode.DoubleRow
ALU = mybir.AluOpType

B = 8
S = 4096
D = 256
P = 128

NST = S // P          # 32 query stripes of 128 rows
NSB = S // 512        # 8 blocks of 512 rows (4 stripes)
E_SHIFT = float(8 * np.log(2.0))  # exp() output centering: diagonal -> 2^8
C1 = float(1024.0 / np.log(2.0))  # bit-exp scale (fp16 mantissa bits)
C2 = float(15 * 1024 - 0.0430 * 1024)  # fp16 exp bias - Schraudolph correction

# k-slices for the scores PSUM tiles: 1536-wide tiles (3 banks each, 2 bufs)
K_SLICES = [(0, 1536), (1536, 1536), (3072, 1024)]



def build_kernel(finalize: bool = True) -> bass.Bass:
    nc = bacc.Bacc(None)

    x_in = nc.declare_dram_parameter("x", [S, D], F32, isOutput=False)
    m_in = nc.declare_dram_parameter("mask", [S, D], F32, isOutput=False)
    out_ext = nc.declare_dram_parameter("out", [1, D], F32, isOutput=True)

    x_in_t = x_in.rearrange("(a p) d -> p a d", p=P)      # [128, 32, 256]
    m_in_t = m_in.rearrange("(a p) d -> p a d", p=P)

    with tile.TileContext(nc) as tc:
        with (
            tc.tile_pool(name="dram", bufs=1, space="DRAM") as dramp,
            tc.tile_pool(name="res", bufs=1) as res,
            tc.tile_pool(name="etile", bufs=8) as ep,
            tc.tile_pool(name="small", bufs=8) as smallp,
            tc.tile_pool(name="ps_scores", bufs=2, space="PSUM") as pss,
            tc.tile_pool(name="ps_misc", bufs=2, space="PSUM") as psm,
        ):
            # DRAM bounce buffers (fp16) for the XBAR transposes
            x16d = dramp.tile([S, D], F16)
            m16d = dramp.tile([S, D], F16)

            # SBUF residents
            xT16 = res.tile([P, 2, S], F16)   # x^T fp16 [d%128, d//128, s]
            mT16 = res.tile([P, 2, S], F16)
            qT8 = res.tile([P, 2, S], F8)     # (x*m/4)^T fp8
            xT8 = res.tile([P, 2, S], F8)     # (x/4)^T fp8
            x16 = res.tile([P, NST, D], F16)  # x fp16 row layout
            dprod = res.tile([P, 2, S], F16)  # qT8*xT8 elementwise
            diag_sb = res.tile([P, 2, 512], F32)  # diag strips (partitions 0/32/64/96)
            bias_all = res.tile([P, NST], F32)    # E_SHIFT - diag
            bias2_all = res.tile([P, NST], F32)   # bias_all*C1 + C2 (bit-exp)
            ones = res.tile([P, 1], F32)
            ones16 = res.tile([P, 1], F16)
            sbW = res.tile([P, 2, 512], F32)  # colsum strips copied from PSUM
            wtot16 = res.tile([P, NST], F16)
            out_sb = res.tile([1, D], F32)

            nc.vector.memset(ones[:], 1.0)
            nc.vector.memset(ones16[:], 1.0)

            # ---- Phase A ----
            NCH = 8  # 512-row chunks
            rows_per = S // NCH
            a_per = rows_per // P  # stripes per chunk in the [p, a, d] view
            xv = x16d.rearrange("(a p) d -> p a d", p=P)
            mv = m16d.rearrange("(a p) d -> p a d", p=P)
            for ch in range(NCH):
                a0 = ch * a_per
                rows = slice(ch * rows_per, (ch + 1) * rows_per)
                # cast f32 -> f16 during DMA (SWDGE); flat contiguous row range
                nc.gpsimd.dma_start(x16d[rows, :], x_in[rows, :])
                nc.gpsimd.dma_start(m16d[rows, :], m_in[rows, :])
                # XBAR transposes: x on the sync ring, m on the scalar ring
                for d in range(2):
                    nc.sync.dma_start(
                        xT16[:, d, rows], x16d[rows, d * P : (d + 1) * P], transpose=True
                    )
                    nc.scalar.dma_start(
                        mT16[:, d, rows], m16d[rows, d * P : (d + 1) * P], transpose=True
                    )
                # row-layout fp16 x for the final matvec
                nc.sync.dma_start(x16[:, a0 : a0 + a_per, :], xv[:, a0 : a0 + a_per, :])

                # fp8 operands: qT8 = (xT/4)*mT, xT8 = xT/4
                nc.vector.scalar_tensor_tensor(
                    out=qT8[:, :, rows], in0=xT16[:, :, rows], scalar=0.25,
                    in1=mT16[:, :, rows], op0=ALU.mult, op1=ALU.mult,
                )
                nc.vector.tensor_scalar(
                    xT8[:, :, rows], xT16[:, :, rows], 0.25, None, ALU.mult
                )
                # dprod for the diagonal: qT8*xT8 summed over d gives s_qq
                nc.vector.tensor_tensor(
                    dprod[:, :, rows], qT8[:, :, rows], xT8[:, :, rows], ALU.mult
                )
                # PE ones-matvec over partitions: diag strip for this 512-chunk
                strip = ch  # 0..7
                g, c = strip // 4, strip % 4
                pd = psm.tile([P, 512], F32, tag="a")
                for d in range(2):
                    nc.tensor.matmul(
                        pd[32 * c : 32 * c + 1, :],
                        lhsT=ones16[:],
                        rhs=dprod[:, d, rows],
                        start=(d == 0),
                        stop=(d == 1),
                        tile_position=(0, 32 * c),
                        skip_group_check=True,
                    )
                nc.vector.tensor_copy(diag_sb[:, g, :][32 * c : 32 * c + 1, :], pd[32 * c : 32 * c + 1, :])

            # transpose diag strips into [128, 32] partition layout:
            # w-chunk i (queries 128i..128i+127) lives at diag_sb[g][32c, t0:t0+128]
            diagP = psm.tile([P, NST], F32, tag="a")
            for i in range(NST):
                g, c, t0 = i // 16, (i % 16) // 4, (i % 4) * P
                nc.tensor.matmul(
                    diagP[:, i : i + 1],
                    lhsT=diag_sb[:, g, t0 : t0 + P][32 * c : 32 * c + 1, :],
                    rhs=ones[32 * c : 32 * c + 1, :],
                    start=True,
                    stop=True,
                    tile_position=(32 * c, 0),
                )
            # bias_all = E_SHIFT - diag ; bias2 = bias_all*C1 + C2
            nc.vector.tensor_scalar(
                bias_all[:], diagP[:], -1.0, E_SHIFT, ALU.mult, ALU.add
            )
            nc.vector.tensor_scalar(
                bias2_all[:], bias_all[:], C1, C2, ALU.mult, ALU.add
            )

            # persistent colsum accumulators (2 banks, 4 col-strips each)
            W0 = psm.tile([P, 512], F32, tag="a")
            W1 = psm.tile([P, 512], F32, tag="a")
            Wt = (W0, W1)

            def emit_colsum(blk, e_tiles, rb):
                first = blk == 0
                last = blk == NSB - 1
                for j in range(4):
                    for g in range(2):
                        for c in range(4):
                            ks = g * 4 + c
                            nc.tensor.matmul(
                                Wt[g][32 * c : 32 * c + 1, :],
                                lhsT=rb[:, j : j + 1],
                                rhs=e_tiles[j][:, ks * 512 : (ks + 1) * 512],
                                start=(first and j == 0),
                                stop=(last and j == 3),
                                tile_position=(0, 32 * c),
                                skip_group_check=True,
                            )

            # ---- Phase B ----
            prev = None
            for blk in range(NSB):
                e_tiles = []
                zpb = smallp.tile([P, 4, 3], F32, tag="z")
                rb = smallp.tile([P, 4], F16, tag="r")
                for j in range(4):
                    qs = blk * 4 + j
                    et = ep.tile([P, S], F16, tag="e")
                    for ksl, (k0, kn) in enumerate(K_SLICES):
                        ps = pss.tile([P, 1536], F32, tag="s")
                        for n in range(kn // 512):
                            nc.tensor.matmul(
                                ps[:, n * 512 : (n + 1) * 512],
                                lhsT=qT8[:, :, qs * P : (qs + 1) * P],
                                rhs=xT8[:, :, k0 + n * 512 : k0 + (n + 1) * 512],
                                start=True,
                                stop=True,
                                perf_mode=DR,
                            )
                        if ksl == 2:
                            # DVE slice: bit-trick exp into the uint16 view
                            nc.vector.tensor_scalar(
                                et[:, k0 : k0 + kn].bitcast(U16),
                                ps[:, :kn],
                                C1,
                                bias2_all[:, qs : qs + 1],
                                ALU.mult,
                                ALU.add,
                            )
                            nc.vector.tensor_reduce(
                                zpb[:, j, :][:, ksl : ksl + 1],
                                et[:, k0 : k0 + kn],
                                mybir.AxisListType.X,
                                ALU.add,
                            )
                        else:
                            nc.scalar.activation(
                                out=et[:, k0 : k0 + kn],
                                in_=ps[:, :kn],
                                func=mybir.ActivationFunctionType.Exp,
                                bias=bias_all[:, qs : qs + 1],
                                scale=1.0,
                                accum_out=zpb[:, j, :][:, ksl : ksl + 1],
                            )
                    e_tiles.append(et)
                    # keep ScalarE fed: defer previous block's colsums
                    if j == 0 and prev is not None:
                        emit_colsum(blk - 1, *prev)
                        prev = None
                # finalize r for the block: r = fp16(1/(sum of 3 partials))
                zs = smallp.tile([P, 4], F32, tag="zs")
                nc.vector.tensor_reduce(zs[:], zpb[:], mybir.AxisListType.X, ALU.add)
                nc.vector.reciprocal(zs[:], zs[:])
                nc.vector.tensor_copy(rb[:], zs[:])
                prev = (e_tiles, rb)
            emit_colsum(NSB - 1, *prev)

            # ---- Tail ----
            for g in range(2):
                nc.vector.tensor_copy(sbW[:, g, :], Wt[g][:])
            wtotP = psm.tile([P, NST], F32, tag="a")
            for i in range(NST):
                g, c, t0 = i // 16, (i % 16) // 4, (i % 4) * P
                nc.tensor.matmul(
                    wtotP[:, i : i + 1],
                    lhsT=sbW[:, g, t0 : t0 + P][32 * c : 32 * c + 1, :],
                    rhs=ones[32 * c : 32 * c + 1, :],
                    start=True,
                    stop=True,
                    tile_position=(32 * c, 0),
                )
            nc.vector.tensor_copy(wtot16[:], wtotP[:])
            po = psm.tile([1, D], F32, tag="a")
            for c in range(NST):
                nc.tensor.matmul(
                    po[:],
                    lhsT=wtot16[:, c : c + 1],
                    rhs=x16[:, c, :],
                    start=(c == 0),
                    stop=(c == NST - 1),
                )
            nc.vector.tensor_copy(out_sb[:], po[:])
            nc.sync.dma_start(out_ext[:, :], out_sb[:])

    if finalize:
        nc.finalize()
    return nc


def _run(x: np.ndarray, drop_mask: np.ndarray, trace: bool = False, nc=None):
    if nc is None:
        nc = build_kernel()
    in_maps = [{"x": x[b], "mask": drop_mask[b]} for b in range(B)]
    res = run_bass_kernel_spmd(nc, in_maps, list(range(B)), trace=trace)
    out = np.stack([res.results[b]["out"].reshape(D) for b in range(B)])
    return out.astype(np.float32), res


def kernel(**inputs: np.ndarray) -> np.ndarray:
    x = np.ascontiguousarray(inputs["x"], dtype=np.float32)
    drop_mask = np.ascontiguousarray(inputs["drop_mask"], dtype=np.float32)
    assert x.shape == (B, S, D) and drop_mask.shape == (B, S, D)
    out, _ = _run(x, drop_mask)
    return out


def profile(**inputs: np.ndarray):
    x = np.ascontiguousarray(inputs["x"], dtype=np.float32)
    drop_mask = np.ascontiguousarray(inputs["drop_mask"], dtype=np.float32)
    out, res = _run(x, drop_mask, trace=True)
    return res.exec_time_ns


if __name__ == "__main__":
    rng = np.random.default_rng(0)
    x = rng.standard_normal((B, S, D)).astype(np.float32)
    m = (rng.random((B, S, D)) < 0.5).astype(np.float32) * 2.0
    out = kernel(x=x, drop_mask=m)
    print(out.shape, out.dtype)
